# revision 19
# baseline (speedup 1.0000x reference)
"""Trainium2 Bass kernel for nn_ChannelAttention (channel attention over att_features).

Reference computation (per batch b):
    x      = att_features[b].T                      # (C=2048, L=196)
    ca     = x @ Wc + bc                            # (C, H=512)
    ph     = h[b] @ Wh + bh                         # (H,)
    scores = tanh(ca + ph) @ wa + ba                # (C,)
    weights= softmax(scores)                        # (C,)   [ba cancels in softmax]
    weighted = weights @ x                          # (L,)
    returns (weighted, weights)

Mapping (pure data parallel: batch sharded over 8 cores, 16 batches/core,
processed in 4 groups of 4 batches, software-pipelined across groups):
  - att host-staged as zero-padded fp16 (BS, 208, 2048) -- fp16 over bf16
    because every tensor here is unit-scale (8x finer mantissa, same cost).
    Loaded twice per batch:
    natural layout (L on partitions) as main-matmul rhs, and via one batched
    DMA-xbar transpose per batch (C on partitions) for the final matvec.
  - Main einsum on TensorE: ca^T (H on partitions, C free) = Wc^T @ att.
  - tanh + (ph+bh+bc) bias fused into one ScalarE activation per PSUM tile
    (bias is per-partition since H is the partition dim). ScalarE is the
    bottleneck engine (~128 x (1024+172)/1.2GHz of tanh work per core).
  - scores = wa . tanh: M=1 matmuls, 4 batches packed concurrently via
    tile_position col-tiling -> psum partitions {0,32,64,96}; hm-outer
    accumulation so only the last H-chunk trails the final tanh.
  - softmax per group on VectorE/ScalarE; no max-subtraction needed
    (|scores| <= sum|wa| bounds exp well inside fp32 range).
  - weighted: weights compacted+transposed via xbar, then col-tiled M=1
    matvecs against the transposed att; deferred one group so the PE stream
    stays dense while the softmax chain completes.
"""

import numpy as np
from contextlib import ExitStack

from concourse import bacc, tile, mybir
from concourse.bass_utils import run_bass_kernel_spmd

F32 = mybir.dt.float32
DT16 = mybir.dt.float16  # fp16: all on-chip data is unit-scale, so fp16's
# 11-bit mantissa beats bf16 at identical cost (PE/DVE/xbar all 16-bit paths)

B, L, C = 128, 196, 2048
H, D = 512, 1024
NCORES = 8
BS = B // NCORES          # 16 batches per core
LP = 208                  # L padded to a multiple of 16 (xbar transpose constraint)
NG = BS // 4              # batch groups of 4 per core

_CACHED_NC = None
LAST_RESULT = None        # BassKernelResults of the most recent run (for test harness)


def _build_kernel(ctx: ExitStack, tc):
    nc = tc.nc
    Tanh = mybir.ActivationFunctionType.Tanh
    Exp = mybir.ActivationFunctionType.Exp
    ADD = mybir.AluOpType.add
    MULT = mybir.AluOpType.mult

    att_d = nc.dram_tensor("att", [BS, LP, C], DT16, kind="ExternalInput").ap()
    ht_d = nc.dram_tensor("ht", [D, BS], DT16, kind="ExternalInput").ap()
    wc_d = nc.dram_tensor("wc", [LP, H], DT16, kind="ExternalInput").ap()
    wh_d = nc.dram_tensor("wh", [D, H], DT16, kind="ExternalInput").ap()
    bhr_d = nc.dram_tensor("bhr", [128, 4], F32, kind="ExternalInput").ap()
    bcr_d = nc.dram_tensor("bcr", [128, 4], F32, kind="ExternalInput").ap()
    war_d = nc.dram_tensor("war", [128, 4], DT16, kind="ExternalInput").ap()
    outw_d = nc.dram_tensor("weighted", [BS, L], F32, kind="ExternalOutput").ap()
    outa_d = nc.dram_tensor("weights", [BS, C], F32, kind="ExternalOutput").ap()

    const = ctx.enter_context(tc.tile_pool(name="const", bufs=1))
    setup = ctx.enter_context(tc.tile_pool(name="setup", bufs=1))

    # ---- setup loads (emission order = SP HWDGE ring order) ----
    wh_t = setup.tile([128, 8, H], DT16, tag="wh")
    nc.sync.dma_start(wh_t[:], wh_d.rearrange("(ko p) h -> p ko h", p=128))
    ht_t = setup.tile([128, 8, BS], DT16, tag="htt")
    nc.sync.dma_start(ht_t[:], ht_d.rearrange("(ko p) b -> p ko b", p=128))
    wc1 = const.tile([128, H], DT16, tag="wc1")
    wc2 = const.tile([80, H], DT16, tag="wc2")
    nc.sync.dma_start(wc1[:], wc_d[0:128, :])
    nc.sync.dma_start(wc2[:], wc_d[128:LP, :])
    war_t = const.tile([128, 4], DT16, tag="war")
    nc.sync.dma_start(war_t[:], war_d[:])
    bhr_t = const.tile([128, 4], F32, tag="bhr")
    bcr_t = const.tile([128, 4], F32, tag="bcr")
    nc.sync.dma_start(bhr_t[:], bhr_d[:])
    nc.sync.dma_start(bcr_t[:], bcr_d[:])

    bhc_t = const.tile([128, 4], F32, tag="bhc")
    phb_t = const.tile([128, 4, BS], F32, tag="phb")

    # Dummy activation up front: walrus places the ACT table load (~2.7us)
    # before it, so the load overlaps the initial DMAs instead of delaying
    # the first real tanh.
    warm_t = const.tile([128, 1], F32, tag="warm")
    nc.vector.memset(warm_t[:], 0.0)
    nc.scalar.activation(warm_t[:], warm_t[:], Tanh)

    def emit_ph(small_ps):
        """ph^T = (Wh^T @ h^T) + bh + bc; (128, BS) per H-chunk, H on partitions."""
        nc.vector.tensor_tensor(bhc_t[:], bhr_t[:], bcr_t[:], ADD)
        for hm in range(4):
            ph_ps = small_ps.tile([128, BS], F32, tag="sps", name=f"phps{hm}")
            for ko in range(8):
                nc.tensor.matmul(
                    ph_ps[:],
                    wh_t[:, ko, 128 * hm : 128 * (hm + 1)],
                    ht_t[:, ko, :],
                    start=(ko == 0),
                    stop=(ko == 7),
                )
            nc.vector.tensor_scalar(
                phb_t[:, hm, :], ph_ps[:], bhc_t[:, hm : hm + 1], None, ADD
            )

    # ---------------- pools for the main pipeline --------------------------------
    attp = ctx.enter_context(tc.tile_pool(name="attp", bufs=5))
    attTp = ctx.enter_context(tc.tile_pool(name="attTp", bufs=5))
    tanhp = ctx.enter_context(tc.tile_pool(name="tanhp", bufs=20))
    grp = ctx.enter_context(tc.tile_pool(name="grp", bufs=2))
    main_ps = ctx.enter_context(tc.tile_pool(name="main_ps", bufs=2, space="PSUM"))
    small_ps = ctx.enter_context(tc.tile_pool(name="small_ps", bufs=4, space="PSUM"))

    def load_aT(gg):
        aTs = []
        for j in range(4):
            aT = attTp.tile([128, 16, LP], DT16, tag="aT", name=f"aT_g{gg}_{j}")
            nc.sync.dma_start(aT[:], att_d[4 * gg + j, :, :], transpose=True)
            aTs.append(aT)
        return aTs

    pending_weighted = None  # (g, wT) deferred so PE isn't blocked on the softmax
    pending_outs = []        # output DMAs deferred behind the next group's loads

    def emit_weighted(g, aTs, wT):
        """Phase D for group g: weighted = weights . att (col-tiled matvecs)."""
        wt_ps = small_ps.tile([128, 512], F32, tag="sps", name=f"wtps{g}")
        for ci in range(16):
            for j in range(4):
                nc.tensor.matmul(
                    wt_ps[32 * j : 32 * j + 1, 0:LP],
                    wT[:, ci, j : j + 1],
                    aTs[j][:, ci, :],
                    start=(ci == 0),
                    stop=(ci == 15),
                    tile_position=(0, 32 * j),
                )
        wsum = grp.tile([128, LP], F32, tag="wsum", name=f"wsum{g}", bufs=3)
        nc.vector.tensor_copy(wsum[:], wt_ps[:, 0:LP])
        pending_outs.append((
            outw_d[4 * g : 4 * g + 4, :],
            wsum[:].rearrange("(a b) n -> a b n", b=32)[:, 0, 0:L]))

    for g in range(NG):
        # ---- natural-layout loads first (critical path for the main matmul) ----
        a1s, a2s = [], []
        for j in range(4):
            bi = 4 * g + j
            a1 = attp.tile([128, C], DT16, tag="a1", name=f"a1_{bi}")
            a2 = attp.tile([80, C], DT16, tag="a2", name=f"a2_{bi}")
            nc.sync.dma_start(a1[:], att_d[bi, 0:128, :])
            nc.sync.dma_start(a2[:], att_d[bi, 128:LP, :])
            a1s.append(a1)
            a2s.append(a2)

        # deferred output DMAs + previous group's transposed-att loads
        for dst, srcap in pending_outs:
            nc.sync.dma_start(dst, srcap)
        pending_outs.clear()
        prev_aTs = None
        prev_wT = None
        if pending_weighted is not None:
            prev_aTs = load_aT(g - 1)
            prev_wT = grp.tile([128, 16, 16], DT16, tag="wT", name=f"wT{g - 1}")
            nc.sync.dma_start(prev_wT[:], pending_weighted[1][:], transpose=True)

        wdense = grp.tile([16, C], DT16, tag="wdense", name=f"wdense{g}")
        nc.vector.memset(wdense[:], 0.0)

        # ---- phase A: main matmul + fused bias/tanh ----
        tanh_t = [[None] * 4 for _ in range(4)]  # [j][hm] -> (128, 2048) bf16
        for j in range(4):
            bi = 4 * g + j
            for hm in range(4):
                lhs1 = wc1[:, 128 * hm : 128 * (hm + 1)]
                lhs2 = wc2[:, 128 * hm : 128 * (hm + 1)]
                tt = tanhp.tile([128, C], DT16, tag="tanh", name=f"tanh_{bi}_{hm}")
                tanh_t[j][hm] = tt
                for ch in range(2):
                    mp = main_ps.tile([128, 1024], F32, tag="mp",
                                      name=f"mp_{bi}_{hm}_{ch}")
                    c0 = 1024 * ch
                    nc.tensor.matmul(mp[:, 0:512], lhs1, a1s[j][:, c0 : c0 + 512],
                                     start=True, stop=False)
                    nc.tensor.matmul(mp[:, 512:1024], lhs1, a1s[j][:, c0 + 512 : c0 + 1024],
                                     start=True, stop=False)
                    nc.tensor.matmul(mp[:, 0:512], lhs2, a2s[j][:, c0 : c0 + 512],
                                     start=False, stop=True)
                    nc.tensor.matmul(mp[:, 512:1024], lhs2, a2s[j][:, c0 + 512 : c0 + 1024],
                                     start=False, stop=True)
                    if g == 0 and j == 0 and hm == 0 and ch == 0:
                        emit_ph(small_ps)
                    nc.scalar.activation(
                        tt[:, c0 : c0 + 1024], mp[:], Tanh,
                        bias=phb_t[:, hm, bi : bi + 1], scale=1.0,
                    )

        # ---- deferred phase D of the previous group (keeps PE stream dense) ----
        if pending_weighted is not None:
            emit_weighted(pending_weighted[0], prev_aTs, prev_wT)
            pending_weighted = None

        # ---- phase B: scores = wa . tanh (col-tiled, 4 batches concurrent).
        # hm-outer accumulation: only the last H-chunk trails the final tanh.
        sc_sb = grp.tile([128, C], F32, tag="sc", name=f"sc{g}", bufs=3)
        sc_ps = [small_ps.tile([128, 512], F32, tag="sps", name=f"scps_g{g}_{cq}")
                 for cq in range(4)]
        for hm in range(4):
            for cq in range(4):
                for j in range(4):
                    nc.tensor.matmul(
                        sc_ps[cq][32 * j : 32 * j + 1, :],
                        war_t[:, hm : hm + 1],
                        tanh_t[j][hm][:, 512 * cq : 512 * cq + 512],
                        start=(hm == 0),
                        stop=(hm == 3),
                        tile_position=(0, 32 * j),
                    )
        for cq in range(4):
            nc.vector.tensor_copy(sc_sb[:, 512 * cq : 512 * (cq + 1)], sc_ps[cq][:])

        # ---- phase C: softmax over channels (rows {0,32,64,96} of sc_sb) ----
        sum_g = grp.tile([128, 1], F32, tag="sum", name=f"sum{g}")
        # in-place exp; free-dim sum accumulated in the same instruction
        nc.scalar.activation(sc_sb[:], sc_sb[:], Exp, accum_out=sum_g[:])
        rsum_g = grp.tile([128, 1], F32, tag="rsum", name=f"rsum{g}")
        nc.vector.reciprocal(rsum_g[:], sum_g[:])
        wbf = grp.tile([128, C], DT16, tag="wbf", name=f"wbf{g}")
        nc.vector.tensor_scalar(wbf[:], sc_sb[:], rsum_g[:], None, MULT)
        nc.vector.tensor_scalar(sc_sb[:], sc_sb[:], rsum_g[:], None, MULT)
        pending_outs.append((
            outa_d[4 * g : 4 * g + 4, :],
            sc_sb[:].rearrange("(a b) n -> a b n", b=32)[:, 0, :]))

        # weights -> dense 16-row tile (transposed next iteration, grouped
        # with the att transposes to minimize xbar-mode transitions)
        nc.sync.dma_start(
            wdense[0:4, :],
            wbf[:].rearrange("(a b) n -> a b n", b=32)[:, 0, :])

        pending_weighted = (g, wdense)

    prev_aTs = load_aT(NG - 1)
    wT = grp.tile([128, 16, 16], DT16, tag="wT", name=f"wT{NG - 1}")
    nc.sync.dma_start(wT[:], pending_weighted[1][:], transpose=True)
    emit_weighted(pending_weighted[0], prev_aTs, wT)
    for dst, srcap in pending_outs:
        nc.sync.dma_start(dst, srcap)


def _get_nc():
    global _CACHED_NC
    if _CACHED_NC is None:
        nc = bacc.Bacc("TRN2", target_bir_lowering=False, debug=False)
        with tile.TileContext(nc) as tc, ExitStack() as ctx:
            _build_kernel(ctx, tc)
        nc.compile()
        _CACHED_NC = nc
    return _CACHED_NC


def kernel(att_features, h, Wc, bc, Wh, bh, wa, ba):
    global LAST_RESULT
    att_features = np.asarray(att_features)
    h = np.asarray(h)
    Wc = np.asarray(Wc, dtype=np.float32)
    bc = np.asarray(bc, dtype=np.float32)
    Wh = np.asarray(Wh, dtype=np.float32)
    bh = np.asarray(bh, dtype=np.float32)
    wa = np.asarray(wa, dtype=np.float32)

    # replicated weight staging (layout/dtype prep only)
    wc_s = np.zeros((LP, H), dtype=np.float16)
    wc_s[:L] = Wc.astype(np.float16)
    wh_s = np.ascontiguousarray(Wh.astype(np.float16))
    bhr_s = np.ascontiguousarray(bh.reshape(4, 128).T)
    bcr_s = np.ascontiguousarray(bc.reshape(4, 128).T)
    war_s = np.ascontiguousarray(wa.reshape(4, 128).T.astype(np.float16))

    in_maps = []
    for m in range(NCORES):
        sl = slice(m * BS, (m + 1) * BS)
        att_s = np.zeros((BS, LP, C), dtype=np.float16)
        att_s[:, :L, :] = att_features[sl].astype(np.float16)
        ht_s = np.ascontiguousarray(h[sl].T.astype(np.float16))
        in_maps.append({
            "att": att_s, "ht": ht_s, "wc": wc_s, "wh": wh_s,
            "bhr": bhr_s, "bcr": bcr_s, "war": war_s,
        })

    nc = _get_nc()
    LAST_RESULT = run_bass_kernel_spmd(nc, in_maps, list(range(NCORES)))
    res = LAST_RESULT.results

    weighted = np.concatenate([res[m]["weighted"] for m in range(NCORES)], axis=0)
    weights = np.concatenate([res[m]["weights"] for m in range(NCORES)], axis=0)
    return weighted.astype(np.float32), weights.astype(np.float32)


# revision 20
# speedup vs baseline: 1.0107x; 1.0107x over previous
"""Trainium2 Bass kernel for nn_ChannelAttention (channel attention over att_features).

Reference computation (per batch b):
    x      = att_features[b].T                      # (C=2048, L=196)
    ca     = x @ Wc + bc                            # (C, H=512)
    ph     = h[b] @ Wh + bh                         # (H,)
    scores = tanh(ca + ph) @ wa + ba                # (C,)
    weights= softmax(scores)                        # (C,)   [ba cancels in softmax]
    weighted = weights @ x                          # (L,)
    returns (weighted, weights)

Mapping (pure data parallel: batch sharded over 8 cores, 16 batches/core,
processed in 4 groups of 4 batches, software-pipelined across groups):
  - att host-staged as zero-padded fp16 (BS, 208, 2048) -- fp16 over bf16
    because every tensor here is unit-scale (8x finer mantissa, same cost).
    Loaded twice per batch:
    natural layout (L on partitions) as main-matmul rhs, and via one batched
    DMA-xbar transpose per batch (C on partitions) for the final matvec.
  - Main einsum on TensorE: ca^T (H on partitions, C free) = Wc^T @ att.
  - tanh + (ph+bh+bc) bias fused into one ScalarE activation per PSUM tile
    (bias is per-partition since H is the partition dim). ScalarE is the
    bottleneck engine (~128 x (1024+172)/1.2GHz of tanh work per core).
  - scores = wa . tanh: M=1 matmuls, 4 batches packed concurrently via
    tile_position col-tiling -> psum partitions {0,32,64,96}; hm-outer
    accumulation so only the last H-chunk trails the final tanh.
  - softmax per group on VectorE/ScalarE; no max-subtraction needed
    (|scores| <= sum|wa| bounds exp well inside fp32 range).
  - weighted: weights compacted+transposed via xbar, then col-tiled M=1
    matvecs against the transposed att; deferred one group so the PE stream
    stays dense while the softmax chain completes.
"""

import numpy as np
from contextlib import ExitStack

from concourse import bacc, tile, mybir
from concourse.bass_utils import run_bass_kernel_spmd

F32 = mybir.dt.float32
DT16 = mybir.dt.float16  # fp16: all on-chip data is unit-scale, so fp16's
# 11-bit mantissa beats bf16 at identical cost (PE/DVE/xbar all 16-bit paths)

B, L, C = 128, 196, 2048
H, D = 512, 1024
NCORES = 8
BS = B // NCORES          # 16 batches per core
LP = 208                  # L padded to a multiple of 16 (xbar transpose constraint)
NG = BS // 4              # batch groups of 4 per core

_CACHED_NC = None
LAST_RESULT = None        # BassKernelResults of the most recent run (for test harness)


def _build_kernel(ctx: ExitStack, tc):
    nc = tc.nc
    Tanh = mybir.ActivationFunctionType.Tanh
    Exp = mybir.ActivationFunctionType.Exp
    ADD = mybir.AluOpType.add
    MULT = mybir.AluOpType.mult

    att_d = nc.dram_tensor("att", [BS, LP, C], DT16, kind="ExternalInput").ap()
    ht_d = nc.dram_tensor("ht", [D, BS], DT16, kind="ExternalInput").ap()
    wc_d = nc.dram_tensor("wc", [LP, H], DT16, kind="ExternalInput").ap()
    wh_d = nc.dram_tensor("wh", [D, H], DT16, kind="ExternalInput").ap()
    bhr_d = nc.dram_tensor("bhr", [128, 4], F32, kind="ExternalInput").ap()
    bcr_d = nc.dram_tensor("bcr", [128, 4], F32, kind="ExternalInput").ap()
    war_d = nc.dram_tensor("war", [128, 4], DT16, kind="ExternalInput").ap()
    outw_d = nc.dram_tensor("weighted", [BS, L], F32, kind="ExternalOutput").ap()
    outa_d = nc.dram_tensor("weights", [BS, C], F32, kind="ExternalOutput").ap()

    const = ctx.enter_context(tc.tile_pool(name="const", bufs=1))
    setup = ctx.enter_context(tc.tile_pool(name="setup", bufs=1))

    # ---- setup loads (emission order = SP HWDGE ring order) ----
    wh_t = setup.tile([128, 8, H], DT16, tag="wh")
    nc.sync.dma_start(wh_t[:], wh_d.rearrange("(ko p) h -> p ko h", p=128))
    ht_t = setup.tile([128, 8, BS], DT16, tag="htt")
    nc.sync.dma_start(ht_t[:], ht_d.rearrange("(ko p) b -> p ko b", p=128))
    wc1 = const.tile([128, H], DT16, tag="wc1")
    wc2 = const.tile([80, H], DT16, tag="wc2")
    nc.sync.dma_start(wc1[:], wc_d[0:128, :])
    nc.sync.dma_start(wc2[:], wc_d[128:LP, :])
    war_t = const.tile([128, 4], DT16, tag="war")
    nc.sync.dma_start(war_t[:], war_d[:])
    bhr_t = const.tile([128, 4], F32, tag="bhr")
    bcr_t = const.tile([128, 4], F32, tag="bcr")
    nc.sync.dma_start(bhr_t[:], bhr_d[:])
    nc.sync.dma_start(bcr_t[:], bcr_d[:])

    bhc_t = const.tile([128, 4], F32, tag="bhc")
    phb_t = const.tile([128, 4, BS], F32, tag="phb")

    # Dummy activation up front: walrus places the ACT table load (~2.7us)
    # before it, so the load overlaps the initial DMAs instead of delaying
    # the first real tanh.
    warm_t = const.tile([128, 1], F32, tag="warm")
    nc.vector.memset(warm_t[:], 0.0)
    nc.scalar.activation(warm_t[:], warm_t[:], Tanh)

    def emit_ph(small_ps):
        """ph^T = (Wh^T @ h^T) + bh + bc; (128, BS) per H-chunk, H on partitions."""
        nc.vector.tensor_tensor(bhc_t[:], bhr_t[:], bcr_t[:], ADD)
        for hm in range(4):
            ph_ps = small_ps.tile([128, BS], F32, tag="sps", name=f"phps{hm}")
            for ko in range(8):
                nc.tensor.matmul(
                    ph_ps[:],
                    wh_t[:, ko, 128 * hm : 128 * (hm + 1)],
                    ht_t[:, ko, :],
                    start=(ko == 0),
                    stop=(ko == 7),
                )
            nc.vector.tensor_scalar(
                phb_t[:, hm, :], ph_ps[:], bhc_t[:, hm : hm + 1], None, ADD
            )

    # ---------------- pools for the main pipeline --------------------------------
    attp = ctx.enter_context(tc.tile_pool(name="attp", bufs=5))
    attTp = ctx.enter_context(tc.tile_pool(name="attTp", bufs=5))
    tanhp = ctx.enter_context(tc.tile_pool(name="tanhp", bufs=20))
    grp = ctx.enter_context(tc.tile_pool(name="grp", bufs=2))
    main_ps = ctx.enter_context(tc.tile_pool(name="main_ps", bufs=2, space="PSUM"))
    small_ps = ctx.enter_context(tc.tile_pool(name="small_ps", bufs=4, space="PSUM"))

    def load_aT(gg):
        aTs = []
        for j in range(4):
            aT = attTp.tile([128, 16, LP], DT16, tag="aT", name=f"aT_g{gg}_{j}")
            nc.sync.dma_start(aT[:], att_d[4 * gg + j, :, :], transpose=True)
            aTs.append(aT)
        return aTs

    pending_weighted = None  # (g, wT) deferred so PE isn't blocked on the softmax
    pending_outs = []        # output DMAs deferred behind the next group's loads

    def emit_weighted(g, aTs, wT):
        """Phase D for group g: weighted = weights . att (col-tiled matvecs)."""
        wt_ps = small_ps.tile([128, 512], F32, tag="sps", name=f"wtps{g}")
        for ci in range(16):
            for j in range(4):
                nc.tensor.matmul(
                    wt_ps[32 * j : 32 * j + 1, 0:LP],
                    wT[:, ci, j : j + 1],
                    aTs[j][:, ci, :],
                    start=(ci == 0),
                    stop=(ci == 15),
                    tile_position=(0, 32 * j),
                )
        wsum = grp.tile([128, LP], F32, tag="wsum", name=f"wsum{g}", bufs=3)
        nc.vector.tensor_copy(wsum[:], wt_ps[:, 0:LP])
        pending_outs.append((
            outw_d[4 * g : 4 * g + 4, :],
            wsum[:].rearrange("(a b) n -> a b n", b=32)[:, 0, 0:L]))

    for g in range(NG):
        # ---- natural-layout loads first (critical path for the main matmul) ----
        a1s, a2s = [], []
        for j in range(4):
            bi = 4 * g + j
            a1 = attp.tile([128, C], DT16, tag="a1", name=f"a1_{bi}")
            a2 = attp.tile([80, C], DT16, tag="a2", name=f"a2_{bi}")
            nc.sync.dma_start(a1[:], att_d[bi, 0:128, :])
            nc.sync.dma_start(a2[:], att_d[bi, 128:LP, :])
            a1s.append(a1)
            a2s.append(a2)

        # deferred output DMAs + previous group's transposed-att loads
        for dst, srcap in pending_outs:
            nc.sync.dma_start(dst, srcap)
        pending_outs.clear()
        prev_aTs = None
        prev_wT = None
        if pending_weighted is not None:
            prev_aTs = load_aT(g - 1)
            prev_wT = grp.tile([128, 16, 16], DT16, tag="wT", name=f"wT{g - 1}")
            nc.sync.dma_start(prev_wT[:], pending_weighted[1][:], transpose=True)

        wdense = grp.tile([16, C], DT16, tag="wdense", name=f"wdense{g}")
        nc.vector.memset(wdense[:], 0.0)

        # ---- phase A: main matmul + fused bias/tanh ----
        tanh_t = [[None] * 4 for _ in range(4)]  # [j][hm] -> (128, 2048) bf16
        for j in range(4):
            bi = 4 * g + j
            for hm in range(4):
                lhs1 = wc1[:, 128 * hm : 128 * (hm + 1)]
                lhs2 = wc2[:, 128 * hm : 128 * (hm + 1)]
                tt = tanhp.tile([128, C], DT16, tag="tanh", name=f"tanh_{bi}_{hm}")
                tanh_t[j][hm] = tt
                for ch in range(2):
                    mp = main_ps.tile([128, 1024], F32, tag="mp",
                                      name=f"mp_{bi}_{hm}_{ch}")
                    c0 = 1024 * ch
                    nc.tensor.matmul(mp[:, 0:512], lhs1, a1s[j][:, c0 : c0 + 512],
                                     start=True, stop=False)
                    nc.tensor.matmul(mp[:, 512:1024], lhs1, a1s[j][:, c0 + 512 : c0 + 1024],
                                     start=True, stop=False)
                    nc.tensor.matmul(mp[:, 0:512], lhs2, a2s[j][:, c0 : c0 + 512],
                                     start=False, stop=True)
                    nc.tensor.matmul(mp[:, 512:1024], lhs2, a2s[j][:, c0 + 512 : c0 + 1024],
                                     start=False, stop=True)
                    if g == 0 and j == 0 and hm == 0 and ch == 0:
                        emit_ph(small_ps)
                    nc.scalar.activation(
                        tt[:, c0 : c0 + 1024], mp[:], Tanh,
                        bias=phb_t[:, hm, bi : bi + 1], scale=1.0,
                    )

        # ---- deferred phase D of the previous group (keeps PE stream dense) ----
        if pending_weighted is not None:
            emit_weighted(pending_weighted[0], prev_aTs, prev_wT)
            pending_weighted = None

        # ---- phase B: scores = wa . tanh (col-tiled, 4 batches concurrent).
        # hm-outer accumulation: only the last H-chunk trails the final tanh.
        sc_sb = grp.tile([128, C], F32, tag="sc", name=f"sc{g}", bufs=3)
        sc_ps = [small_ps.tile([128, 512], F32, tag="sps", name=f"scps_g{g}_{cq}")
                 for cq in range(4)]
        for hm in range(4):
            for cq in range(4):
                for j in range(4):
                    nc.tensor.matmul(
                        sc_ps[cq][32 * j : 32 * j + 1, :],
                        war_t[:, hm : hm + 1],
                        tanh_t[j][hm][:, 512 * cq : 512 * cq + 512],
                        start=(hm == 0),
                        stop=(hm == 3),
                        tile_position=(0, 32 * j),
                    )
        for cq in range(4):
            nc.vector.tensor_copy(sc_sb[:, 512 * cq : 512 * (cq + 1)], sc_ps[cq][:])

        # ---- phase C: softmax over channels (rows {0,32,64,96} of sc_sb) ----
        # in-place exp in two halves (first half starts as soon as its score
        # chunks land), free-dim sums fused into the activations via accum_out
        s2 = grp.tile([128, 2], F32, tag="sum", name=f"sum{g}")
        nc.scalar.activation(sc_sb[:, 0:1024], sc_sb[:, 0:1024], Exp,
                             accum_out=s2[:, 0:1])
        nc.scalar.activation(sc_sb[:, 1024:2048], sc_sb[:, 1024:2048], Exp,
                             accum_out=s2[:, 1:2])
        sum_g = grp.tile([128, 1], F32, tag="sumt", name=f"sumt{g}")
        nc.vector.tensor_tensor(sum_g[:], s2[:, 0:1], s2[:, 1:2], ADD)
        rsum_g = grp.tile([128, 1], F32, tag="rsum", name=f"rsum{g}")
        nc.vector.reciprocal(rsum_g[:], sum_g[:])
        wbf = grp.tile([128, C], DT16, tag="wbf", name=f"wbf{g}")
        nc.vector.tensor_scalar(wbf[:], sc_sb[:], rsum_g[:], None, MULT)
        nc.vector.tensor_scalar(sc_sb[:], sc_sb[:], rsum_g[:], None, MULT)
        pending_outs.append((
            outa_d[4 * g : 4 * g + 4, :],
            sc_sb[:].rearrange("(a b) n -> a b n", b=32)[:, 0, :]))

        # weights -> dense 16-row tile (transposed next iteration, grouped
        # with the att transposes to minimize xbar-mode transitions)
        nc.sync.dma_start(
            wdense[0:4, :],
            wbf[:].rearrange("(a b) n -> a b n", b=32)[:, 0, :])

        pending_weighted = (g, wdense)

    prev_aTs = load_aT(NG - 1)
    wT = grp.tile([128, 16, 16], DT16, tag="wT", name=f"wT{NG - 1}")
    nc.sync.dma_start(wT[:], pending_weighted[1][:], transpose=True)
    emit_weighted(pending_weighted[0], prev_aTs, wT)
    for dst, srcap in pending_outs:
        nc.sync.dma_start(dst, srcap)


def _get_nc():
    global _CACHED_NC
    if _CACHED_NC is None:
        nc = bacc.Bacc("TRN2", target_bir_lowering=False, debug=False)
        with tile.TileContext(nc) as tc, ExitStack() as ctx:
            _build_kernel(ctx, tc)
        nc.compile()
        _CACHED_NC = nc
    return _CACHED_NC


def kernel(att_features, h, Wc, bc, Wh, bh, wa, ba):
    global LAST_RESULT
    att_features = np.asarray(att_features)
    h = np.asarray(h)
    Wc = np.asarray(Wc, dtype=np.float32)
    bc = np.asarray(bc, dtype=np.float32)
    Wh = np.asarray(Wh, dtype=np.float32)
    bh = np.asarray(bh, dtype=np.float32)
    wa = np.asarray(wa, dtype=np.float32)

    # replicated weight staging (layout/dtype prep only)
    wc_s = np.zeros((LP, H), dtype=np.float16)
    wc_s[:L] = Wc.astype(np.float16)
    wh_s = np.ascontiguousarray(Wh.astype(np.float16))
    bhr_s = np.ascontiguousarray(bh.reshape(4, 128).T)
    bcr_s = np.ascontiguousarray(bc.reshape(4, 128).T)
    war_s = np.ascontiguousarray(wa.reshape(4, 128).T.astype(np.float16))

    in_maps = []
    for m in range(NCORES):
        sl = slice(m * BS, (m + 1) * BS)
        att_s = np.zeros((BS, LP, C), dtype=np.float16)
        att_s[:, :L, :] = att_features[sl].astype(np.float16)
        ht_s = np.ascontiguousarray(h[sl].T.astype(np.float16))
        in_maps.append({
            "att": att_s, "ht": ht_s, "wc": wc_s, "wh": wh_s,
            "bhr": bhr_s, "bcr": bcr_s, "war": war_s,
        })

    nc = _get_nc()
    LAST_RESULT = run_bass_kernel_spmd(nc, in_maps, list(range(NCORES)))
    res = LAST_RESULT.results

    weighted = np.concatenate([res[m]["weighted"] for m in range(NCORES)], axis=0)
    weights = np.concatenate([res[m]["weights"] for m in range(NCORES)], axis=0)
    return weighted.astype(np.float32), weights.astype(np.float32)
